# revision 2
# baseline (speedup 1.0000x reference)
"""Grouped MoE MLP (64 experts) on 8 Trainium2 NeuronCores.

Strategy: expert parallelism. Each core owns 8 experts (size-sorted "snake"
assignment so every core gets the same per-slot padded token capacity and the
padding is tight). Host pre-transposes w1 to [H, F], casts weights/activations
to bf16, and gathers each core's tokens into per-expert padded blocks laid out
transposed ([H, tokens]) so both matmuls stream tokens as the moving operand:

    hT[f, t]   = w1t[e] (stationary, [h,f] tiles) @ xT (moving, [h, t])
    hT         = gelu(hT)                     (ScalarE, PSUM f32 -> SBUF bf16)
    outT[o, t] = w2[e] (stationary, [f,o] tiles) @ hT (moving, [f, t])

Weights stream HBM->SBUF once per core (double-buffered), PSUM accumulates in
f32, output is written back f32 transposed and un-permuted on host.
"""

import numpy as np

NCORES = 8
SLOTS = 8  # experts per core
NE = 64
H = 1024
F = 2048
T = 16384
P = 128
KO = H // P  # 8  k-tiles for mm1 (contraction over H)
FO = F // P  # 16 f-tiles (mm1 output tiles / mm2 contraction)
OO = H // P  # 8  output h-tiles for mm2
NMAX = 512  # max moving-operand length (one fp32 PSUM bank)

ACT_FN = "Gelu"  # overridable for CoreSim tests (Gelu not implemented there)

_prog_cache = {}


def _build_program(C):
    """Build the SPMD Bass program for per-slot token capacities C (len SLOTS)."""
    from contextlib import ExitStack

    import concourse.tile as tile
    from concourse import bacc, mybir
    from concourse.bass import MemorySpace

    bf16 = mybir.dt.bfloat16
    f32 = mybir.dt.float32
    CTOT = int(sum(C))
    Cmax = int(max(C))
    CmaxB = min(Cmax, NMAX)  # chunked tile width

    nc = bacc.Bacc("TRN2", target_bir_lowering=False, debug=False, num_devices=NCORES)
    w1t_d = nc.dram_tensor("w1t", [SLOTS, H, F], bf16, kind="ExternalInput").ap()
    w2_d = nc.dram_tensor("w2", [SLOTS, F, H], bf16, kind="ExternalInput").ap()
    xT_d = nc.dram_tensor("xT", [H, CTOT], bf16, kind="ExternalInput").ap()
    outT_d = nc.dram_tensor("outT", [H, CTOT], f32, kind="ExternalOutput").ap()

    w1t_r = w1t_d.rearrange("s (ho hi) f -> s hi ho f", hi=P)  # [S,128,KO,F]
    w2_r = w2_d.rearrange("s (fo fi) h -> s fi fo h", fi=P)  # [S,128,FO,H]
    xT_r = xT_d.rearrange("(ho hi) t -> hi ho t", hi=P)  # [128,KO,CTOT]
    outT_r = outT_d.rearrange("(oo oi) t -> oi oo t", oi=P)  # [128,OO,CTOT]

    with tile.TileContext(nc) as tc, ExitStack() as ctx:
        w1_pool = ctx.enter_context(tc.tile_pool(name="w1", bufs=2))
        w2_pool = ctx.enter_context(tc.tile_pool(name="w2", bufs=2))
        x_pool = ctx.enter_context(tc.tile_pool(name="x", bufs=2))
        h_pool = ctx.enter_context(tc.tile_pool(name="h", bufs=2))
        o_pool = ctx.enter_context(tc.tile_pool(name="o", bufs=4))
        ph_pool = ctx.enter_context(
            tc.tile_pool(name="ph", bufs=3, space=MemorySpace.PSUM)
        )
        po_pool = ctx.enter_context(
            tc.tile_pool(name="po", bufs=3, space=MemorySpace.PSUM)
        )

        off = 0
        for j in range(SLOTS):
            Cj = int(C[j])
            fch = min(512, F)
            hch = min(512, H)
            w1_sb = w1_pool.tile([P, KO, F], bf16, tag="w1")
            for fc in range(0, F, fch):  # chunked for pipelining
                nc.sync.dma_start(
                    w1_sb[:, :, fc : fc + fch], w1t_r[j, :, :, fc : fc + fch]
                )
            w2_sb = w2_pool.tile([P, FO, H], bf16, tag="w2")
            for hc in range(0, H, hch):
                nc.sync.dma_start(
                    w2_sb[:, :, hc : hc + hch], w2_r[j, :, :, hc : hc + hch]
                )
            x_sb = x_pool.tile([P, KO, Cmax], bf16, tag="x")
            nc.sync.dma_start(x_sb[:, :, :Cj], xT_r[:, :, off : off + Cj])

            for nb in range(0, Cj, NMAX):
                NB = min(NMAX, Cj - nb)
                h_sb = h_pool.tile([P, FO, CmaxB], bf16, tag="h")
                for fo in range(FO):
                    ph = ph_pool.tile([P, NMAX], f32, tag="ph")
                    for k in range(KO):
                        nc.tensor.matmul(
                            ph[:, :NB],
                            w1_sb[:, k, fo * P : (fo + 1) * P],
                            x_sb[:, k, nb : nb + NB],
                            start=(k == 0),
                            stop=(k == KO - 1),
                        )
                    nc.scalar.activation(
                        h_sb[:, fo, :NB],
                        ph[:, :NB],
                        getattr(mybir.ActivationFunctionType, ACT_FN),
                    )
                for oo in range(OO):
                    po = po_pool.tile([P, NMAX], f32, tag="po")
                    for fo in range(FO):
                        nc.tensor.matmul(
                            po[:, :NB],
                            w2_sb[:, fo, oo * P : (oo + 1) * P],
                            h_sb[:, fo, :NB],
                            start=(fo == 0),
                            stop=(fo == FO - 1),
                        )
                    o_sb = o_pool.tile([P, NMAX], f32, tag="o")
                    nc.vector.tensor_copy(o_sb[:, :NB], po[:, :NB])
                    nc.sync.dma_start(
                        outT_r[:, oo, off + nb : off + nb + NB], o_sb[:, :NB]
                    )
            off += Cj

    nc.compile()
    return nc


def _get_program(C):
    key = tuple(int(c) for c in C)
    if key not in _prog_cache:
        _prog_cache[key] = _build_program(key)
    return _prog_cache[key]


def plan(sizes):
    """Expert->core/slot assignment + slot capacities from token counts."""
    sizes = np.asarray(sizes, np.int64)
    assert sizes.shape == (NE,) and sizes.sum() == T
    order = np.argsort(-sizes, kind="stable")  # descending
    # expert_of[core][slot]
    expert_of = [[int(order[s * NCORES + c]) for s in range(SLOTS)] for c in range(NCORES)]
    C = []
    for s in range(SLOTS):
        m = max(int(sizes[order[s * NCORES + c]]) for c in range(NCORES))
        C.append(max(16, -(-m // 8) * 8))  # round up to multiple of 8, min 16
    offs = np.concatenate([[0], np.cumsum(C)]).astype(np.int64)
    return expert_of, C, offs


def prepare_inputs(x, w1, w2, sizes, expert_of, C, offs):
    """Host-side shard/pad/transpose/cast. Returns per-core input maps."""
    import ml_dtypes

    bf16 = ml_dtypes.bfloat16
    x = np.asarray(x, np.float32)
    tok_offs = np.concatenate([[0], np.cumsum(sizes)]).astype(np.int64)
    w1_bf = np.asarray(w1, np.float32).astype(bf16)  # [NE, F, H]
    w2_bf = np.asarray(w2, np.float32).astype(bf16)  # [NE, F, H]
    CTOT = int(sum(C))

    in_maps = []
    for c in range(NCORES):
        experts = expert_of[c]
        w1t_c = np.ascontiguousarray(
            w1_bf[experts].transpose(0, 2, 1)
        )  # [S, H, F] bf16
        w2_c = np.ascontiguousarray(w2_bf[experts])  # [S, F, H] bf16
        xc = np.zeros((CTOT, H), np.float32)
        for s, e in enumerate(experts):
            n = int(sizes[e])
            xc[offs[s] : offs[s] + n] = x[tok_offs[e] : tok_offs[e] + n]
        xT_c = np.ascontiguousarray(xc.T).astype(bf16)  # [H, CTOT] bf16
        in_maps.append({"w1t": w1t_c, "w2": w2_c, "xT": xT_c})
    return in_maps


def scatter_output(results, sizes, expert_of, offs):
    """Gather per-core transposed outputs back into the full [T, H] f32 output."""
    tok_offs = np.concatenate([[0], np.cumsum(sizes)]).astype(np.int64)
    out = np.empty((T, H), np.float32)
    for c in range(NCORES):
        outT_c = np.asarray(results[c]["outT"])  # [H, CTOT] f32
        for s, e in enumerate(expert_of[c]):
            n = int(sizes[e])
            out[tok_offs[e] : tok_offs[e] + n] = outT_c[:, offs[s] : offs[s] + n].T
    return out


LAST_RUN = None  # BassKernelResults from the most recent kernel() call


def kernel(x, w1, w2, tokens_per_expert):
    global LAST_RUN
    from concourse import bass_utils

    sizes = np.asarray(tokens_per_expert, np.int64)
    expert_of, C, offs = plan(sizes)
    nc = _get_program(C)
    in_maps = prepare_inputs(x, w1, w2, sizes, expert_of, C, offs)
    res = bass_utils.run_bass_kernel_spmd(nc, in_maps, core_ids=list(range(NCORES)))
    LAST_RUN = res
    return scatter_output(res.results, sizes, expert_of, offs)



# revision 3
# speedup vs baseline: 1.2933x; 1.2933x over previous
"""Grouped MoE MLP (64 experts) on 8 Trainium2 NeuronCores.

Strategy: expert parallelism. Each core owns 8 experts (size-sorted "snake"
assignment so every core gets the same per-slot padded token capacity and the
padding is tight). Host pre-lays-out every tensor so each device DMA is one
large fully-contiguous transfer (>=0.5 MB, 4-32 KB runs per partition):

    w1t[s] : [128 hi, KO*F]   (hi, k, f)   one 4 MB DMA per slot
    w2[s]  : [128 fi, FO*H]   (fi, fo, h)  one 4 MB DMA per slot
    xT[s]  : [128 hi, KO*Cmax] (hi, k, t)  one ~0.6 MB DMA per slot
    outT[s]: [128 oi, OO*Cmax] (oi, oo, t) one ~0.6 MB bf16 DMA per slot

Both matmuls keep weights stationary and stream tokens as the moving operand:

    hT[f, t]   = w1t[e] (stationary, [h,f] tiles) @ xT (moving, [h, t])
    hT         = gelu(hT)                     (ScalarE, PSUM f32 -> SBUF bf16)
    outT[o, t] = w2[e] (stationary, [f,o] tiles) @ hT (moving, [f, t])

Weights stream HBM->SBUF once per core (double-buffered), PSUM accumulates in
f32, output is written back bf16 and upcast + un-permuted on host.
"""

import numpy as np

NCORES = 8
SLOTS = 8  # experts per core
NE = 64
H = 1024
F = 2048
T = 16384
P = 128
KO = H // P  # 8  k-tiles for mm1 (contraction over H)
FO = F // P  # 16 f-tiles (mm1 output tiles / mm2 contraction)
OO = H // P  # 8  output h-tiles for mm2
NMAX = 512  # max moving-operand length (one fp32 PSUM bank)

ACT_FN = "Gelu"  # overridable for CoreSim tests (Gelu not implemented there)

_prog_cache = {}


def _build_program(C):
    """Build the SPMD Bass program for per-slot token capacities C (len SLOTS)."""
    from contextlib import ExitStack

    import concourse.tile as tile
    from concourse import bacc, mybir
    from concourse.bass import MemorySpace

    bf16 = mybir.dt.bfloat16
    f32 = mybir.dt.float32
    Cmax = int(max(C))
    CmaxB = min(Cmax, NMAX)  # chunked tile width

    nc = bacc.Bacc("TRN2", target_bir_lowering=False, debug=False, num_devices=NCORES)
    w1t_d = nc.dram_tensor("w1t", [SLOTS, P, KO * F], bf16, kind="ExternalInput").ap()
    w2_d = nc.dram_tensor("w2", [SLOTS, P, FO * H], bf16, kind="ExternalInput").ap()
    xT_d = nc.dram_tensor("xT", [SLOTS, P, KO * Cmax], bf16, kind="ExternalInput").ap()
    outT_d = nc.dram_tensor(
        "outT", [SLOTS, P, OO * Cmax], bf16, kind="ExternalOutput"
    ).ap()

    with tile.TileContext(nc) as tc, ExitStack() as ctx:
        w1_pool = ctx.enter_context(tc.tile_pool(name="w1", bufs=2))
        w2_pool = ctx.enter_context(tc.tile_pool(name="w2", bufs=2))
        x_pool = ctx.enter_context(tc.tile_pool(name="x", bufs=2))
        h_pool = ctx.enter_context(tc.tile_pool(name="h", bufs=2))
        o_pool = ctx.enter_context(tc.tile_pool(name="o", bufs=2))
        ph_pool = ctx.enter_context(
            tc.tile_pool(name="ph", bufs=3, space=MemorySpace.PSUM)
        )
        po_pool = ctx.enter_context(
            tc.tile_pool(name="po", bufs=3, space=MemorySpace.PSUM)
        )

        for j in range(SLOTS):
            Cj = int(C[j])
            x_sb = x_pool.tile([P, KO * Cmax], bf16, tag="x")
            nc.sync.dma_start(x_sb, xT_d[j])
            w1_sb = w1_pool.tile([P, KO * F], bf16, tag="w1")
            nc.sync.dma_start(w1_sb, w1t_d[j])
            w2_sb = w2_pool.tile([P, FO * H], bf16, tag="w2")
            nc.sync.dma_start(w2_sb, w2_d[j])
            o_sb = o_pool.tile([P, OO * Cmax], bf16, tag="o")

            for nb in range(0, Cj, NMAX):
                NB = min(NMAX, Cj - nb)
                h_sb = h_pool.tile([P, FO * CmaxB], bf16, tag="h")
                for fo in range(FO):
                    ph = ph_pool.tile([P, NMAX], f32, tag="ph")
                    for k in range(KO):
                        nc.tensor.matmul(
                            ph[:, :NB],
                            w1_sb[:, k * F + fo * P : k * F + (fo + 1) * P],
                            x_sb[:, k * Cmax + nb : k * Cmax + nb + NB],
                            start=(k == 0),
                            stop=(k == KO - 1),
                        )
                    nc.scalar.activation(
                        h_sb[:, fo * CmaxB : fo * CmaxB + NB],
                        ph[:, :NB],
                        getattr(mybir.ActivationFunctionType, ACT_FN),
                    )
                for oo in range(OO):
                    po = po_pool.tile([P, NMAX], f32, tag="po")
                    for fo in range(FO):
                        nc.tensor.matmul(
                            po[:, :NB],
                            w2_sb[:, fo * H + oo * P : fo * H + (oo + 1) * P],
                            h_sb[:, fo * CmaxB : fo * CmaxB + NB],
                            start=(fo == 0),
                            stop=(fo == FO - 1),
                        )
                    nc.vector.tensor_copy(
                        o_sb[:, oo * Cmax + nb : oo * Cmax + nb + NB], po[:, :NB]
                    )
            nc.sync.dma_start(outT_d[j], o_sb)

    nc.compile()
    return nc


def _get_program(C):
    key = tuple(int(c) for c in C)
    if key not in _prog_cache:
        _prog_cache[key] = _build_program(key)
    return _prog_cache[key]


def plan(sizes):
    """Expert->core/slot assignment + slot capacities from token counts."""
    sizes = np.asarray(sizes, np.int64)
    assert sizes.shape == (NE,) and sizes.sum() == T
    order = np.argsort(-sizes, kind="stable")  # descending
    # expert_of[core][slot]
    expert_of = [[int(order[s * NCORES + c]) for s in range(SLOTS)] for c in range(NCORES)]
    C = []
    for s in range(SLOTS):
        m = max(int(sizes[order[s * NCORES + c]]) for c in range(NCORES))
        C.append(max(16, -(-m // 8) * 8))  # round up to multiple of 8, min 16
    return expert_of, C


def prepare_inputs(x, w1, w2, sizes, expert_of, C):
    """Host-side shard/pad/transpose/cast. Returns per-core input maps."""
    import ml_dtypes

    bf16 = ml_dtypes.bfloat16
    x = np.asarray(x, np.float32)
    tok_offs = np.concatenate([[0], np.cumsum(sizes)]).astype(np.int64)
    w1_bf = np.asarray(w1, np.float32).astype(bf16)  # [NE, F, H]
    w2_bf = np.asarray(w2, np.float32).astype(bf16)  # [NE, F, H]
    Cmax = int(max(C))

    in_maps = []
    for c in range(NCORES):
        experts = expert_of[c]
        # w1t: [S, hi, k, f] flattened to [S, 128, KO*F]
        w1t_c = np.ascontiguousarray(
            w1_bf[experts].transpose(0, 2, 1).reshape(SLOTS, KO, P, F).transpose(0, 2, 1, 3)
        ).reshape(SLOTS, P, KO * F)
        # w2: [S, fi, fo, h] flattened to [S, 128, FO*H]
        w2_c = np.ascontiguousarray(
            w2_bf[experts].reshape(SLOTS, FO, P, H).transpose(0, 2, 1, 3)
        ).reshape(SLOTS, P, FO * H)
        # xT: [S, hi, k, t] flattened to [S, 128, KO*Cmax]
        xT_c = np.zeros((SLOTS, P, KO, Cmax), np.float32)
        for s, e in enumerate(experts):
            n = int(sizes[e])
            xe = x[tok_offs[e] : tok_offs[e] + n]  # [n, H]
            xT_c[s, :, :, :n] = xe.T.reshape(KO, P, n).transpose(1, 0, 2)
        xT_c = xT_c.reshape(SLOTS, P, KO * Cmax).astype(bf16)
        in_maps.append({"w1t": w1t_c, "w2": w2_c, "xT": xT_c})
    return in_maps


def scatter_output(results, sizes, expert_of, C):
    """Gather per-core transposed outputs back into the full [T, H] f32 output."""
    tok_offs = np.concatenate([[0], np.cumsum(sizes)]).astype(np.int64)
    Cmax = int(max(C))
    out = np.empty((T, H), np.float32)
    for c in range(NCORES):
        # [S, oi, oo, t] -> per expert [H, n] -> [n, H]
        outT_c = np.asarray(results[c]["outT"]).reshape(SLOTS, P, OO, Cmax)
        for s, e in enumerate(expert_of[c]):
            n = int(sizes[e])
            blk = outT_c[s, :, :, :n].astype(np.float32)  # [oi, oo, n]
            out[tok_offs[e] : tok_offs[e] + n] = (
                blk.transpose(1, 0, 2).reshape(H, n).T
            )
    return out


LAST_RUN = None  # BassKernelResults from the most recent kernel() call


def kernel(x, w1, w2, tokens_per_expert):
    global LAST_RUN
    from concourse import bass_utils

    sizes = np.asarray(tokens_per_expert, np.int64)
    expert_of, C = plan(sizes)
    nc = _get_program(C)
    in_maps = prepare_inputs(x, w1, w2, sizes, expert_of, C)
    res = bass_utils.run_bass_kernel_spmd(nc, in_maps, core_ids=list(range(NCORES)))
    LAST_RUN = res
    return scatter_output(res.results, sizes, expert_of, C)
